# revision 9
# baseline (speedup 1.0000x reference)
"""Trainium2 Bass kernel for nn_AttentionBlock (masked GroupNorm + jagged full attention).

Contract: kernel(**inputs) takes FULL unsharded inputs (as in reference.setup_inputs())
and returns the FULL [8, 1024, 512] fp32 output. Internally shards data-parallel over
the batch: sample b -> NeuronCore b (8 cores).

v2: fp8e4 (e4m3) DoubleRow matmuls for qkv/v/av/proj (2x contraction per instr,
0.5 cycles/row), bf16 pair-packed scores (2 concurrent PE row groups), exp on
ScalarE at 1-ktile granularity with ping-pong PSUM buffers so the Activation
engine (the roofline for this kernel, ~1 col/cycle @1.2GHz over H*L*L elements)
never stalls. Elementwise work balanced across DVE (copies, reciprocal,
normalize, residual) and Pool/GpSimd (xn quantize, reciprocal broadcast).

Per-core dataflow (sample s):
  xmT bf16 [C,L] (host-masked, transposed)
    -> GroupNorm stats via bn_stats + group-selector matmuls -> xn8 e4m3 [128,2,L] x2
    -> qkT = W_qk^T @ xn (DoubleRow fp8, PSUM->SBUF bf16 + bias on DVE)
    -> v8  = xn^T @ W_v  (DoubleRow fp8, masked rows + denominator mask-row)
    -> per head pair: scoresT[k,q] (bf16, 64-row tiles in 2 PE row groups)
    -> exp on ScalarE (scale=1/8 folded) -> e4m3
    -> av^T = [v|mask]^T @ exp (DoubleRow fp8, row 64 = softmax denominator)
    -> reciprocal (DVE) -> partition_broadcast (Pool) -> normalize (DVE) -> e4m3
    -> proj (DoubleRow fp8) + host-prepared residual (x*mask + folded biases), DMA out.

Padded tokens: x masked to zero on host => xn for padded tokens ~= -mean*rstd*gamma
(tiny since group means ~0), v rows explicitly zeroed, denominator mask-row zero.
Padded q columns produce ~uniform attention over valid k (output ~1e-2 abs, well
under the correctness gate); residual rows are zero so outputs there stay tiny.
"""

import numpy as np
import ml_dtypes
from contextlib import ExitStack

B, L, C, G, H = 8, 1024, 512, 32, 8
DH = C // H          # 64
CPG = C // G         # 16
EPS = 1e-5
NT = L // 128        # 8 token tiles
CT = C // 128        # 4 channel tiles
CT2 = C // 256       # 2 DoubleRow channel groups
QC = L // 512        # 2 query chunks

BF16 = ml_dtypes.bfloat16
E4M3 = ml_dtypes.float8_e4m3

_CACHE = {}


def _build():
    import concourse.bass as bass
    import concourse.tile as tile
    from concourse import bacc, mybir

    f32 = mybir.dt.float32
    bf16 = mybir.dt.bfloat16
    e4 = mybir.dt.float8e4
    Alu = mybir.AluOpType
    Act = mybir.ActivationFunctionType
    DR = mybir.MatmulPerfMode.DoubleRow

    nc = bacc.Bacc("TRN2", target_bir_lowering=False)

    # ---- per-core DRAM inputs (host-prepped) ----
    xmT_d = nc.dram_tensor("xmT", [C, L], bf16, kind="ExternalInput")
    xm_d = nc.dram_tensor("xmr", [L, C], f32, kind="ExternalInput")   # residual: x*mf + folded biases
    wqk_d = nc.dram_tensor("wqk", [CT2, 8, 128, 2, 128], e4, kind="ExternalInput")  # lhsT (q,k couts)
    wv_d = nc.dram_tensor("wv", [CT2, 128, 2, C], e4, kind="ExternalInput")         # rhs (v couts)
    wp_d = nc.dram_tensor("wp", [CT2, 128, 2, C], e4, kind="ExternalInput")         # Wproj rhs
    bqk_d = nc.dram_tensor("bqk", [128, 8], f32, kind="ExternalInput")              # qk bias per couttile
    gam_d = nc.dram_tensor("gam", [128, CT], f32, kind="ExternalInput")             # gamma per cintile
    vmask_d = nc.dram_tensor("vmask", [128, NT], f32, kind="ExternalInput")         # token mask per ktile
    sel_d = nc.dram_tensor("sel", [CT, 128, G], f32, kind="ExternalInput")          # chan->group selector
    selT_d = nc.dram_tensor("selT", [G, C], f32, kind="ExternalInput")              # group->chan selector
    icnt_d = nc.dram_tensor("icnt", [G, 1], f32, kind="ExternalInput")              # 1/(len*cpg)
    out_d = nc.dram_tensor("out", [L, C], f32, kind="ExternalOutput")

    with tile.TileContext(nc) as tc, ExitStack() as ctx:
        pc = ctx.enter_context(tc.tile_pool(name="consts", bufs=1))
        pb = ctx.enter_context(tc.tile_pool(name="big", bufs=1))
        ps = ctx.enter_context(tc.tile_pool(name="psum", bufs=1, space="PSUM"))

        # ---- constant / input loads (small consts first: they gate GroupNorm) ----
        bqk_sb = pc.tile([128, 8], f32, tag="bqk", name="bqk")
        gam_sb = pc.tile([128, CT], f32, tag="gam", name="gam")
        vmask_sb = pc.tile([128, NT], f32, tag="vmask", name="vmask")
        selT_sb = pc.tile([G, C], f32, tag="selT", name="selT")
        icnt_sb = pc.tile([G, 1], f32, tag="icnt", name="icnt")
        for t_sb, t_d in [(bqk_sb, bqk_d), (gam_sb, gam_d), (vmask_sb, vmask_d),
                          (selT_sb, selT_d), (icnt_sb, icnt_d)]:
            nc.sync.dma_start(t_sb[:], t_d[:, :])
        sel_sb = [pc.tile([128, G], f32, tag=f"sel{ct}", name=f"sel{ct}") for ct in range(CT)]
        for ct in range(CT):
            nc.sync.dma_start(sel_sb[ct][:], sel_d[ct])
        xmt_sb = [pb.tile([128, L], bf16, tag=f"xmT{t}", name=f"xmT{t}") for t in range(CT)]
        for t in range(CT):
            nc.sync.dma_start(xmt_sb[t][:], xmT_d[128 * t:128 * (t + 1), :])
        wqk_sb = [[pc.tile([128, 2, 128], e4, tag=f"wqk{c2}_{ot}", name=f"wqk{c2}_{ot}")
                   for ot in range(8)] for c2 in range(CT2)]
        for c2 in range(CT2):
            for ot in range(8):
                nc.sync.dma_start(wqk_sb[c2][ot][:], wqk_d[c2, ot])
        wv_sb = [pc.tile([128, 2, C], e4, tag=f"wv{c2}", name=f"wv{c2}") for c2 in range(CT2)]
        wp_sb = [pc.tile([128, 2, C], e4, tag=f"wp{c2}", name=f"wp{c2}") for c2 in range(CT2)]
        for c2 in range(CT2):
            nc.sync.dma_start(wv_sb[c2][:], wv_d[c2])
            nc.sync.dma_start(wp_sb[c2][:], wp_d[c2])

        # ---- Phase 1: GroupNorm (stats over valid tokens; zeros from host masking) ----
        smm = [pb.tile([128, 2], f32, tag=f"smm{t}", name=f"smm{t}") for t in range(CT)]
        ps_g = ps.tile([G, 2], f32, tag="pA", name="psg")
        for t in range(CT):
            bns = pb.tile([128, 2, 6], f32, tag="bns", name="bns")
            nc.vector.bn_stats(bns[:, 0, :], xmt_sb[t][:, 0:512])
            nc.vector.bn_stats(bns[:, 1, :], xmt_sb[t][:, 512:1024])
            mv = pb.tile([128, 2], f32, tag="mv", name="mv")
            nc.vector.bn_aggr(mv[:], bns[:])
            sq = pb.tile([128, 1], f32, tag="sq", name="sq")
            nc.vector.tensor_mul(sq[:], mv[:, 0:1], mv[:, 0:1])
            # smm = [sum(x), sum(x^2)] recovered from mean/var over all 1024 (incl. zeros)
            nc.vector.tensor_scalar(smm[t][:, 0:1], mv[:, 0:1], float(L), None, Alu.mult)
            nc.vector.tensor_scalar(smm[t][:, 1:2], mv[:, 1:2], sq[:, 0:1], float(L), Alu.add, Alu.mult)
        for t in range(CT):
            nc.tensor.matmul(ps_g[:], sel_sb[t][:], smm[t][:], start=(t == 0), stop=(t == CT - 1))
        grp = pb.tile([G, 2], f32, tag="grp", name="grp")      # [mean_g, rstd_g]
        ex2 = pb.tile([G, 1], f32, tag="ex2", name="ex2")
        nc.vector.tensor_scalar(grp[:, 0:1], ps_g[:, 0:1], icnt_sb[:, 0:1], None, Alu.mult)
        nc.vector.tensor_scalar(ex2[:], ps_g[:, 1:2], icnt_sb[:, 0:1], None, Alu.mult)
        mm2 = pb.tile([G, 1], f32, tag="mm2", name="mm2")
        nc.vector.tensor_mul(mm2[:], grp[:, 0:1], grp[:, 0:1])
        var = pb.tile([G, 1], f32, tag="var", name="var")
        nc.vector.tensor_tensor(var[:], ex2[:], mm2[:], Alu.subtract)
        sd = pb.tile([G, 1], f32, tag="sd", name="sd")
        eps_sb = pb.tile([G, 1], f32, tag="eps", name="eps")
        nc.vector.memset(eps_sb[:], EPS)
        nc.scalar.activation(sd[:], var[:], Act.Sqrt, bias=eps_sb[:], scale=1.0)
        nc.vector.reciprocal(grp[:, 1:2], sd[:])

        # xn8[c2][p, i, tok] = xn(channel 256*c2 + 128*i + p, tok) in e4m3
        xn8_sb = [pb.tile([128, 2, L], e4, tag=f"xn8_{c2}", name=f"xn8_{c2}") for c2 in range(CT2)]
        rg_sb = pb.tile([128, CT], f32, tag="rg", name="rg")
        chst = [pb.tile([128, 2], f32, tag=f"chst{t}", name=f"chst{t}") for t in range(CT)]
        for t in range(CT):
            ps_b = ps.tile([128, 2], f32, tag="pB", name="psb")
            nc.tensor.matmul(ps_b[:], selT_sb[:, 128 * t:128 * (t + 1)], grp[:], start=True, stop=True)
            nc.vector.tensor_copy(chst[t][:], ps_b[:])
            nc.vector.tensor_mul(rg_sb[:, t:t + 1], chst[t][:, 1:2], gam_sb[:, t:t + 1])
            nc.gpsimd.tensor_scalar(xn8_sb[t // 2][:, t % 2, :], xmt_sb[t][:],
                                    chst[t][:, 0:1], rg_sb[:, t:t + 1],
                                    Alu.subtract, Alu.mult)

        qkT_sb = [pb.tile([128, L], bf16, tag=f"qkT{ot}", name=f"qkT{ot}") for ot in range(8)]
        # v8[p, g, u, h, d]: v value of token 128*(2g+u)+p, head h, dh d; d==64 is the
        # mask row; padded to stride 72 so the DoubleRow ldweights step is 16B-aligned
        v8_sb = pb.tile([128, NT // 2, 2, H, 72], e4, tag="v8", name="v8")
        attn8_sb = [pb.tile([128, 2, L], e4, tag=f"attn8_{c2}", name=f"attn8_{c2}") for c2 in range(CT2)]
        bcast_sb = [pb.tile([64, 512], f32, tag=f"bcast{j}", name=f"bcast{j}", bufs=2) for j in range(2)]

        def emit_qk(ot):
            for qc in range(QC):
                qs = slice(512 * qc, 512 * (qc + 1))
                pq = ps.tile([128, 512], f32, tag=("pA" if qc == 0 else "pB"), name="pq")
                for c2 in range(CT2):
                    nc.tensor.matmul(pq[:], wqk_sb[c2][ot][:], xn8_sb[c2][:, :, qs],
                                     start=(c2 == 0), stop=(c2 == CT2 - 1), perf_mode=DR)
                nc.vector.tensor_scalar(qkT_sb[ot][:, qs], pq[:],
                                        bqk_sb[:, ot:ot + 1], None, Alu.add)

        def emit_v():
            for kt in range(NT):
                pv = ps.tile([128, 512], f32, tag=("pA" if kt % 2 == 0 else "pB"), name="pv")
                for c2 in range(CT2):
                    nc.tensor.matmul(pv[:], xn8_sb[c2][:, :, 128 * kt:128 * (kt + 1)], wv_sb[c2][:],
                                     start=(c2 == 0), stop=(c2 == CT2 - 1), perf_mode=DR)
                g, u = kt // 2, kt % 2
                nc.vector.tensor_scalar(v8_sb[:, g, u, :, 0:DH],
                                        pv[:].rearrange("p (h d) -> p h d", h=H),
                                        vmask_sb[:, kt:kt + 1], None, Alu.mult)
                nc.vector.tensor_copy(v8_sb[:, g, u, :, DH],
                                      vmask_sb[:, kt:kt + 1].to_broadcast((128, H)))

        def emit_attn(p, qc):
            kT = qkT_sb[4 + p]
            qT = qkT_sb[p]
            qs = slice(512 * qc, 512 * (qc + 1))
            avs = [ps.tile([DH + 1, 512], f32, tag=("avA" if j == 0 else "avB"), name=f"av{j}")
                   for j in range(2)]
            # expT2[p, j, kt, q]: exp'd transposed scores for the two heads of this pair
            expT2 = pb.tile([128, 2, NT, 512], e4, tag="expT2", name="expT2", bufs=2)
            sX = [None, None]

            def scores(kt):
                ks = slice(128 * kt, 128 * (kt + 1))
                s = ps.tile([128, 2, 512], f32, tag=("sA" if kt % 2 == 0 else "sB"), name="s")
                sX[kt % 2] = s
                nc.tensor.matmul(s[:, 0, :], kT[0:64, ks], qT[0:64, qs], start=True, stop=True)
                nc.tensor.matmul(s[:, 1, :], kT[64:128, ks], qT[64:128, qs], start=True, stop=True)
                return s

            def expk(kt, s):
                nc.scalar.activation(expT2[:, :, kt, :], s[:], Act.Exp, bias=0.0, scale=0.125)

            def av_group(g):
                for j in range(2):
                    h = 2 * p + j
                    nc.tensor.matmul(avs[j][:], v8_sb[:, g, :, h, 0:DH + 1],
                                     expT2[:, j, 2 * g:2 * g + 2, :],
                                     start=(g == 0), stop=(g == NT // 2 - 1), perf_mode=DR)

            # software pipeline: scores a ktile ahead of exp; av trails by 2 ktiles
            for kt in range(NT):
                s = scores(kt)
                expk(kt, s)
                if kt >= 3 and kt % 2 == 1:
                    av_group((kt - 3) // 2)   # g = 0,1,2 at kt = 3,5,7
            av_group(NT // 2 - 1)

            for j in range(2):
                # custom DVE / Pool ops can't touch PSUM: copy the denominator row out first
                den = pb.tile([1, 512], f32, tag=f"den{j}", name=f"den{j}", bufs=2)
                nc.vector.tensor_copy(den[:], avs[j][DH:DH + 1, :])
                rec = pb.tile([1, 512], f32, tag=f"rec{j}", name=f"rec{j}", bufs=2)
                nc.vector.reciprocal_approx_fast(rec[:], den[:])
                nc.gpsimd.partition_broadcast(bcast_sb[j][:], rec[:])
                nc.vector.tensor_tensor(attn8_sb[p // 2][64 * j:64 * (j + 1), p % 2, qs],
                                        avs[j][0:DH, :], bcast_sb[j][:], Alu.mult)

        xm_sb = [pb.tile([128, C], f32, tag=f"xm{t}", name=f"xm{t}") for t in range(NT)]

        def emit_proj(qt):
            po = ps.tile([128, 512], f32, tag=("pA" if qt % 2 == 0 else "pB"), name="po")
            for c2 in range(CT2):
                nc.tensor.matmul(po[:], attn8_sb[c2][:, :, 128 * qt:128 * (qt + 1)], wp_sb[c2][:],
                                 start=(c2 == 0), stop=(c2 == CT2 - 1), perf_mode=DR)
            o_sb = pb.tile([128, C], f32, tag=f"o{qt % 2}", name=f"o{qt % 2}")
            nc.vector.tensor_add(o_sb[:], po[:], xm_sb[qt][:])
            nc.sync.dma_start(out_d[128 * qt:128 * (qt + 1), :], o_sb[:])

        # ---- emission order: front-load pair 0 so the Activation engine starts early;
        # proj for each q-half runs as soon as all pairs finish that half ----
        emit_qk(0)
        emit_qk(4)
        emit_v()
        emit_attn(0, 0)
        emit_qk(1)
        emit_qk(5)
        emit_attn(1, 0)
        emit_qk(2)
        emit_qk(6)
        for t in range(NT):
            nc.sync.dma_start(xm_sb[t][:], xm_d[128 * t:128 * (t + 1), :])
        emit_attn(2, 0)
        emit_qk(3)
        emit_qk(7)
        emit_attn(3, 0)
        for qt in range(NT // 2):
            emit_proj(qt)
        for p in range(CT):
            emit_attn(p, 1)
        for qt in range(NT // 2, NT):
            emit_proj(qt)

    nc.compile()
    return nc


def _get_nc():
    if "nc" not in _CACHE:
        _CACHE["nc"] = _build()
    return _CACHE["nc"]


def _prep_weights(gamma, beta, Wqkv, bqkv, Wproj, bproj):
    """Host-side constant prep shared across cores."""
    W = np.asarray(Wqkv, np.float32)
    bq = np.asarray(bqkv, np.float32) + np.asarray(beta, np.float32) @ W   # fold beta
    Wp = np.asarray(Wproj, np.float32)
    bv = bq[2 * C:3 * C]
    # residual-side constant: bproj + bv @ Wproj (added to masked rows on host)
    resid_bias = np.asarray(bproj, np.float32) + bv @ Wp

    # DoubleRow lhsT tiles: channel (c2, i, p) = 256*c2 + 128*i + p
    wqk = np.zeros((CT2, 8, 128, 2, 128), E4M3)
    for c2 in range(CT2):
        for ot in range(8):
            for i in range(2):
                blk = W[256 * c2 + 128 * i:256 * c2 + 128 * (i + 1),
                        128 * ot:128 * (ot + 1)]
                wqk[c2, ot, :, i, :] = blk.astype(E4M3)
    wv = np.zeros((CT2, 128, 2, C), E4M3)
    wp = np.zeros((CT2, 128, 2, C), E4M3)
    for c2 in range(CT2):
        for i in range(2):
            wv[c2, :, i, :] = W[256 * c2 + 128 * i:256 * c2 + 128 * (i + 1),
                                2 * C:3 * C].astype(E4M3)
            wp[c2, :, i, :] = Wp[256 * c2 + 128 * i:256 * c2 + 128 * (i + 1), :].astype(E4M3)
    bqk = np.zeros((128, 8), np.float32)
    for ot in range(8):
        bqk[:, ot] = bq[128 * ot:128 * (ot + 1)]
    gam = np.asarray(gamma, np.float32).reshape(CT, 128).T.copy()
    sel = np.zeros((CT, 128, G), np.float32)
    for ct in range(CT):
        for c in range(128):
            sel[ct, c, (128 * ct + c) // CPG] = 1.0
    selT = np.zeros((G, C), np.float32)
    for c in range(C):
        selT[c // CPG, c] = 1.0
    return dict(wqk=wqk, wv=wv, wp=wp, bqk=bqk, gam=gam, sel=sel,
                selT=selT), resid_bias


def kernel(x, lengths, gamma, beta, Wqkv, bqkv, Wproj, bproj):
    from concourse.bass_utils import run_bass_kernel_spmd

    x = np.asarray(x, np.float32)
    lengths = np.asarray(lengths).astype(np.int64)
    const, resid_bias = _prep_weights(gamma, beta, Wqkv, bqkv, Wproj, bproj)

    in_maps = []
    for s in range(B):
        ln = int(lengths[s])
        mf = (np.arange(L) < ln).astype(np.float32)
        xm = x[s] * mf[:, None]
        xmr = xm + mf[:, None] * resid_bias[None, :]
        xmT = np.ascontiguousarray(xm.T).astype(BF16)
        vmask = mf.reshape(NT, 128).T.copy()
        icnt = np.full((G, 1), 1.0 / max(ln * CPG, 1), np.float32)
        m = dict(const)
        m.update(xmT=xmT, xmr=xmr, vmask=vmask, icnt=icnt)
        in_maps.append(m)

    nc = _get_nc()
    res = run_bass_kernel_spmd(nc, in_maps, core_ids=list(range(B)))
    _CACHE["last_res"] = res
    out = np.stack([res.results[s]["out"] for s in range(B)], axis=0)
    return out.astype(np.float32)


if __name__ == "__main__":
    rng = np.random.default_rng(0)
    x = rng.standard_normal((B, L, C), dtype=np.float32)
    lengths = rng.integers(L // 2, L + 1, size=(B,))
    gamma = np.ones(C, np.float32)
    beta = np.zeros(C, np.float32)
    Wqkv = (rng.standard_normal((C, 3 * C)) * 0.02).astype(np.float32)
    bqkv = np.zeros(3 * C, np.float32)
    Wproj = (rng.standard_normal((C, C)) * 0.02).astype(np.float32)
    bproj = np.zeros(C, np.float32)
    out = kernel(x=x, lengths=lengths, gamma=gamma, beta=beta, Wqkv=Wqkv,
                 bqkv=bqkv, Wproj=Wproj, bproj=bproj)
    print("out", out.shape, out.dtype, np.abs(out).max())


# revision 10
# speedup vs baseline: 1.4101x; 1.4101x over previous
"""Trainium2 Bass kernel for nn_AttentionBlock (masked GroupNorm + jagged full attention).

Contract: kernel(**inputs) takes FULL unsharded inputs (as in reference.setup_inputs())
and returns the FULL [8, 1024, 512] fp32 output. Internally shards data-parallel over
the batch: sample b -> NeuronCore b (8 cores).

v2: fp8e4 (e4m3) DoubleRow matmuls for qkv/v/av/proj (2x contraction per instr,
0.5 cycles/row), bf16 pair-packed scores (2 concurrent PE row groups), exp on
ScalarE at 1-ktile granularity with ping-pong PSUM buffers so the Activation
engine (the roofline for this kernel, ~1 col/cycle @1.2GHz over H*L*L elements)
never stalls. Elementwise work balanced across DVE (copies, reciprocal,
normalize, residual) and Pool/GpSimd (xn quantize, reciprocal broadcast).

Per-core dataflow (sample s):
  xmT bf16 [C,L] (host-masked, transposed)
    -> GroupNorm stats via bn_stats + group-selector matmuls -> xn8 e4m3 [128,2,L] x2
    -> qkT = W_qk^T @ xn (DoubleRow fp8, PSUM->SBUF bf16 + bias on DVE)
    -> v8  = xn^T @ W_v  (DoubleRow fp8, masked rows + denominator mask-row)
    -> per head pair: scoresT[k,q] (bf16, 64-row tiles in 2 PE row groups)
    -> exp on ScalarE (scale=1/8 folded) -> e4m3
    -> av^T = [v|mask]^T @ exp (DoubleRow fp8, row 64 = softmax denominator)
    -> reciprocal (DVE) -> partition_broadcast (Pool) -> normalize (DVE) -> e4m3
    -> proj (DoubleRow fp8) + host-prepared residual (x*mask + folded biases), DMA out.

Padded tokens: x masked to zero on host => xn for padded tokens ~= -mean*rstd*gamma
(tiny since group means ~0), v rows explicitly zeroed, denominator mask-row zero.
Padded q columns produce ~uniform attention over valid k (output ~1e-2 abs, well
under the correctness gate); residual rows are zero so outputs there stay tiny.
"""

import numpy as np
import ml_dtypes
from contextlib import ExitStack

B, L, C, G, H = 8, 1024, 512, 32, 8
DH = C // H          # 64
CPG = C // G         # 16
EPS = 1e-5
NT = L // 128        # 8 token tiles
CT = C // 128        # 4 channel tiles
CT2 = C // 256       # 2 DoubleRow channel groups
QC = L // 512        # 2 query chunks

BF16 = ml_dtypes.bfloat16
E4M3 = ml_dtypes.float8_e4m3

_CACHE = {}


def _build():
    import concourse.bass as bass
    import concourse.tile as tile
    from concourse import bacc, mybir

    f32 = mybir.dt.float32
    bf16 = mybir.dt.bfloat16
    e4 = mybir.dt.float8e4
    Alu = mybir.AluOpType
    Act = mybir.ActivationFunctionType
    DR = mybir.MatmulPerfMode.DoubleRow

    nc = bacc.Bacc("TRN2", target_bir_lowering=False)

    # ---- per-core DRAM inputs (host-prepped) ----
    xmT_d = nc.dram_tensor("xmT", [C, L], bf16, kind="ExternalInput")
    xm_d = nc.dram_tensor("xmr", [L, C], f32, kind="ExternalInput")   # residual: x*mf + folded biases
    wqk_d = nc.dram_tensor("wqk", [CT2, 8, 128, 2, 128], e4, kind="ExternalInput")  # lhsT (q,k couts)
    wv_d = nc.dram_tensor("wv", [CT2, 128, 2, C], e4, kind="ExternalInput")         # rhs (v couts)
    wp_d = nc.dram_tensor("wp", [CT2, 128, 2, C], e4, kind="ExternalInput")         # Wproj rhs
    bqk_d = nc.dram_tensor("bqk", [128, 8], f32, kind="ExternalInput")              # qk bias per couttile
    gam_d = nc.dram_tensor("gam", [128, CT], f32, kind="ExternalInput")             # gamma per cintile
    vmask_d = nc.dram_tensor("vmask", [128, NT], f32, kind="ExternalInput")         # token mask per ktile
    sel_d = nc.dram_tensor("sel", [CT, 128, G], f32, kind="ExternalInput")          # chan->group selector
    selT_d = nc.dram_tensor("selT", [G, C], f32, kind="ExternalInput")              # group->chan selector
    icnt_d = nc.dram_tensor("icnt", [G, 1], f32, kind="ExternalInput")              # 1/(len*cpg)
    out_d = nc.dram_tensor("out", [L, C], f32, kind="ExternalOutput")

    with tile.TileContext(nc) as tc, ExitStack() as ctx:
        pc = ctx.enter_context(tc.tile_pool(name="consts", bufs=1))
        pb = ctx.enter_context(tc.tile_pool(name="big", bufs=1))
        ps = ctx.enter_context(tc.tile_pool(name="psum", bufs=1, space="PSUM"))

        # ---- constant / input loads (small consts first: they gate GroupNorm) ----
        bqk_sb = pc.tile([128, 8], f32, tag="bqk", name="bqk")
        gam_sb = pc.tile([128, CT], f32, tag="gam", name="gam")
        vmask_sb = pc.tile([128, NT], f32, tag="vmask", name="vmask")
        selT_sb = pc.tile([G, C], f32, tag="selT", name="selT")
        icnt_sb = pc.tile([G, 1], f32, tag="icnt", name="icnt")
        for t_sb, t_d in [(bqk_sb, bqk_d), (gam_sb, gam_d), (vmask_sb, vmask_d),
                          (selT_sb, selT_d), (icnt_sb, icnt_d)]:
            nc.sync.dma_start(t_sb[:], t_d[:, :])
        sel_sb = [pc.tile([128, G], f32, tag=f"sel{ct}", name=f"sel{ct}") for ct in range(CT)]
        for ct in range(CT):
            nc.sync.dma_start(sel_sb[ct][:], sel_d[ct])
        xmt_sb = [pb.tile([128, L], bf16, tag=f"xmT{t}", name=f"xmT{t}") for t in range(CT)]
        for t in range(CT):
            nc.sync.dma_start(xmt_sb[t][:], xmT_d[128 * t:128 * (t + 1), :])
        wqk_sb = [[pc.tile([128, 2, 128], e4, tag=f"wqk{c2}_{ot}", name=f"wqk{c2}_{ot}")
                   for ot in range(8)] for c2 in range(CT2)]
        for c2 in range(CT2):
            for ot in range(8):
                nc.sync.dma_start(wqk_sb[c2][ot][:], wqk_d[c2, ot])
        wv_sb = [pc.tile([128, 2, C], e4, tag=f"wv{c2}", name=f"wv{c2}") for c2 in range(CT2)]
        wp_sb = [pc.tile([128, 2, C], e4, tag=f"wp{c2}", name=f"wp{c2}") for c2 in range(CT2)]
        for c2 in range(CT2):
            nc.sync.dma_start(wv_sb[c2][:], wv_d[c2])
            nc.sync.dma_start(wp_sb[c2][:], wp_d[c2])

        # ---- Phase 1: GroupNorm (stats over valid tokens; zeros from host masking) ----
        smm = [pb.tile([128, 2], f32, tag=f"smm{t}", name=f"smm{t}") for t in range(CT)]
        ps_g = ps.tile([G, 2], f32, tag="pA", name="psg")
        for t in range(CT):
            bns = pb.tile([128, 2, 6], f32, tag="bns", name="bns")
            nc.vector.bn_stats(bns[:, 0, :], xmt_sb[t][:, 0:512])
            nc.vector.bn_stats(bns[:, 1, :], xmt_sb[t][:, 512:1024])
            mv = pb.tile([128, 2], f32, tag="mv", name="mv")
            nc.vector.bn_aggr(mv[:], bns[:])
            sq = pb.tile([128, 1], f32, tag="sq", name="sq")
            nc.vector.tensor_mul(sq[:], mv[:, 0:1], mv[:, 0:1])
            # smm = [sum(x), sum(x^2)] recovered from mean/var over all 1024 (incl. zeros)
            nc.vector.tensor_scalar(smm[t][:, 0:1], mv[:, 0:1], float(L), None, Alu.mult)
            nc.vector.tensor_scalar(smm[t][:, 1:2], mv[:, 1:2], sq[:, 0:1], float(L), Alu.add, Alu.mult)
        for t in range(CT):
            nc.tensor.matmul(ps_g[:], sel_sb[t][:], smm[t][:], start=(t == 0), stop=(t == CT - 1))
        grp = pb.tile([G, 2], f32, tag="grp", name="grp")      # [mean_g, rstd_g]
        ex2 = pb.tile([G, 1], f32, tag="ex2", name="ex2")
        nc.vector.tensor_scalar(grp[:, 0:1], ps_g[:, 0:1], icnt_sb[:, 0:1], None, Alu.mult)
        nc.vector.tensor_scalar(ex2[:], ps_g[:, 1:2], icnt_sb[:, 0:1], None, Alu.mult)
        mm2 = pb.tile([G, 1], f32, tag="mm2", name="mm2")
        nc.vector.tensor_mul(mm2[:], grp[:, 0:1], grp[:, 0:1])
        var = pb.tile([G, 1], f32, tag="var", name="var")
        nc.vector.tensor_tensor(var[:], ex2[:], mm2[:], Alu.subtract)
        sd = pb.tile([G, 1], f32, tag="sd", name="sd")
        eps_sb = pb.tile([G, 1], f32, tag="eps", name="eps")
        nc.vector.memset(eps_sb[:], EPS)
        nc.scalar.activation(sd[:], var[:], Act.Sqrt, bias=eps_sb[:], scale=1.0)
        nc.vector.reciprocal(grp[:, 1:2], sd[:])

        # xn8[c2][p, i, tok] = xn(channel 256*c2 + 128*i + p, tok) in e4m3
        xn8_sb = [pb.tile([128, 2, L], e4, tag=f"xn8_{c2}", name=f"xn8_{c2}") for c2 in range(CT2)]
        rg_sb = pb.tile([128, CT], f32, tag="rg", name="rg")
        chst = [pb.tile([128, 2], f32, tag=f"chst{t}", name=f"chst{t}") for t in range(CT)]
        for t in range(CT):
            ps_b = ps.tile([128, 2], f32, tag="pB", name="psb")
            nc.tensor.matmul(ps_b[:], selT_sb[:, 128 * t:128 * (t + 1)], grp[:], start=True, stop=True)
            nc.vector.tensor_copy(chst[t][:], ps_b[:])
            nc.vector.tensor_mul(rg_sb[:, t:t + 1], chst[t][:, 1:2], gam_sb[:, t:t + 1])
            nc.vector.tensor_scalar(xn8_sb[t // 2][:, t % 2, :], xmt_sb[t][:],
                                    chst[t][:, 0:1], rg_sb[:, t:t + 1],
                                    Alu.subtract, Alu.mult)

        qkT_sb = [pb.tile([128, L], bf16, tag=f"qkT{ot}", name=f"qkT{ot}") for ot in range(8)]
        # v8[p, g, u, h, d]: v value of token 128*(2g+u)+p, head h, dh d; d==64 is the
        # mask row; padded to stride 72 so the DoubleRow ldweights step is 16B-aligned
        v8_sb = pb.tile([128, NT // 2, 2, H, 72], e4, tag="v8", name="v8")
        attn8_sb = [pb.tile([128, 2, L], e4, tag=f"attn8_{c2}", name=f"attn8_{c2}") for c2 in range(CT2)]
        bcast_sb = [pb.tile([64, 512], f32, tag=f"bcast{j}", name=f"bcast{j}", bufs=2) for j in range(2)]

        def emit_qk(ot):
            for qc in range(QC):
                qs = slice(512 * qc, 512 * (qc + 1))
                pq = ps.tile([128, 512], f32, tag=("pA" if qc == 0 else "pB"), name="pq")
                for c2 in range(CT2):
                    nc.tensor.matmul(pq[:], wqk_sb[c2][ot][:], xn8_sb[c2][:, :, qs],
                                     start=(c2 == 0), stop=(c2 == CT2 - 1), perf_mode=DR)
                nc.vector.tensor_scalar(qkT_sb[ot][:, qs], pq[:],
                                        bqk_sb[:, ot:ot + 1], None, Alu.add)

        def emit_v():
            for kt in range(NT):
                pv = ps.tile([128, 512], f32, tag=("pA" if kt % 2 == 0 else "pB"), name="pv")
                for c2 in range(CT2):
                    nc.tensor.matmul(pv[:], xn8_sb[c2][:, :, 128 * kt:128 * (kt + 1)], wv_sb[c2][:],
                                     start=(c2 == 0), stop=(c2 == CT2 - 1), perf_mode=DR)
                g, u = kt // 2, kt % 2
                nc.vector.tensor_scalar(v8_sb[:, g, u, :, 0:DH],
                                        pv[:].rearrange("p (h d) -> p h d", h=H),
                                        vmask_sb[:, kt:kt + 1], None, Alu.mult)
                nc.vector.tensor_copy(v8_sb[:, g, u, :, DH],
                                      vmask_sb[:, kt:kt + 1].to_broadcast((128, H)))

        def emit_attn(p, qc):
            kT = qkT_sb[4 + p]
            qT = qkT_sb[p]
            qs = slice(512 * qc, 512 * (qc + 1))
            avs = [ps.tile([DH + 1, 512], f32, tag=("avA" if j == 0 else "avB"), name=f"av{j}")
                   for j in range(2)]
            # expT2[p, j, kt, q]: exp'd transposed scores for the two heads of this pair
            expT2 = pb.tile([128, 2, NT, 512], e4, tag="expT2", name="expT2", bufs=2)
            sX = [None, None]

            def scores(kt):
                ks = slice(128 * kt, 128 * (kt + 1))
                s = ps.tile([128, 2, 512], f32, tag=("sA" if kt % 2 == 0 else "sB"), name="s")
                sX[kt % 2] = s
                nc.tensor.matmul(s[:, 0, :], kT[0:64, ks], qT[0:64, qs], start=True, stop=True)
                nc.tensor.matmul(s[:, 1, :], kT[64:128, ks], qT[64:128, qs], start=True, stop=True)
                return s

            def expk(kt, s):
                nc.scalar.activation(expT2[:, :, kt, :], s[:], Act.Exp, bias=0.0, scale=0.125)

            def av_group(g):
                for j in range(2):
                    h = 2 * p + j
                    nc.tensor.matmul(avs[j][:], v8_sb[:, g, :, h, 0:DH + 1],
                                     expT2[:, j, 2 * g:2 * g + 2, :],
                                     start=(g == 0), stop=(g == NT // 2 - 1), perf_mode=DR)

            # software pipeline: scores a ktile ahead of exp; av trails by 2 ktiles
            for kt in range(NT):
                s = scores(kt)
                expk(kt, s)
                if kt >= 3 and kt % 2 == 1:
                    av_group((kt - 3) // 2)   # g = 0,1,2 at kt = 3,5,7
            av_group(NT // 2 - 1)

            for j in range(2):
                # custom DVE / Pool ops can't touch PSUM: copy the denominator row out first
                den = pb.tile([1, 512], f32, tag=f"den{j}", name=f"den{j}", bufs=2)
                nc.vector.tensor_copy(den[:], avs[j][DH:DH + 1, :])
                rec = pb.tile([1, 512], f32, tag=f"rec{j}", name=f"rec{j}", bufs=2)
                nc.vector.reciprocal_approx_fast(rec[:], den[:])
                nc.gpsimd.partition_broadcast(bcast_sb[j][:], rec[:])
                nc.vector.tensor_tensor(attn8_sb[p // 2][64 * j:64 * (j + 1), p % 2, qs],
                                        avs[j][0:DH, :], bcast_sb[j][:], Alu.mult)

        xm_sb = [pb.tile([128, C], f32, tag=f"xm{t}", name=f"xm{t}") for t in range(NT)]

        def emit_proj(qt):
            po = ps.tile([128, 512], f32, tag=("pA" if qt % 2 == 0 else "pB"), name="po")
            for c2 in range(CT2):
                nc.tensor.matmul(po[:], attn8_sb[c2][:, :, 128 * qt:128 * (qt + 1)], wp_sb[c2][:],
                                 start=(c2 == 0), stop=(c2 == CT2 - 1), perf_mode=DR)
            o_sb = pb.tile([128, C], f32, tag=f"o{qt % 2}", name=f"o{qt % 2}")
            nc.vector.tensor_add(o_sb[:], po[:], xm_sb[qt][:])
            nc.sync.dma_start(out_d[128 * qt:128 * (qt + 1), :], o_sb[:])

        # ---- emission order: front-load pair 0 so the Activation engine starts early;
        # proj for each q-half runs as soon as all pairs finish that half ----
        emit_qk(0)
        emit_qk(4)
        emit_v()
        emit_attn(0, 0)
        emit_qk(1)
        emit_qk(5)
        emit_attn(1, 0)
        emit_qk(2)
        emit_qk(6)
        for t in range(NT):
            nc.sync.dma_start(xm_sb[t][:], xm_d[128 * t:128 * (t + 1), :])
        emit_attn(2, 0)
        emit_qk(3)
        emit_qk(7)
        emit_attn(3, 0)
        for qt in range(NT // 2):
            emit_proj(qt)
        for p in range(CT):
            emit_attn(p, 1)
        for qt in range(NT // 2, NT):
            emit_proj(qt)

    nc.compile()
    return nc


def _get_nc():
    if "nc" not in _CACHE:
        _CACHE["nc"] = _build()
    return _CACHE["nc"]


def _prep_weights(gamma, beta, Wqkv, bqkv, Wproj, bproj):
    """Host-side constant prep shared across cores."""
    W = np.asarray(Wqkv, np.float32)
    bq = np.asarray(bqkv, np.float32) + np.asarray(beta, np.float32) @ W   # fold beta
    Wp = np.asarray(Wproj, np.float32)
    bv = bq[2 * C:3 * C]
    # residual-side constant: bproj + bv @ Wproj (added to masked rows on host)
    resid_bias = np.asarray(bproj, np.float32) + bv @ Wp

    # DoubleRow lhsT tiles: channel (c2, i, p) = 256*c2 + 128*i + p
    wqk = np.zeros((CT2, 8, 128, 2, 128), E4M3)
    for c2 in range(CT2):
        for ot in range(8):
            for i in range(2):
                blk = W[256 * c2 + 128 * i:256 * c2 + 128 * (i + 1),
                        128 * ot:128 * (ot + 1)]
                wqk[c2, ot, :, i, :] = blk.astype(E4M3)
    wv = np.zeros((CT2, 128, 2, C), E4M3)
    wp = np.zeros((CT2, 128, 2, C), E4M3)
    for c2 in range(CT2):
        for i in range(2):
            wv[c2, :, i, :] = W[256 * c2 + 128 * i:256 * c2 + 128 * (i + 1),
                                2 * C:3 * C].astype(E4M3)
            wp[c2, :, i, :] = Wp[256 * c2 + 128 * i:256 * c2 + 128 * (i + 1), :].astype(E4M3)
    bqk = np.zeros((128, 8), np.float32)
    for ot in range(8):
        bqk[:, ot] = bq[128 * ot:128 * (ot + 1)]
    gam = np.asarray(gamma, np.float32).reshape(CT, 128).T.copy()
    sel = np.zeros((CT, 128, G), np.float32)
    for ct in range(CT):
        for c in range(128):
            sel[ct, c, (128 * ct + c) // CPG] = 1.0
    selT = np.zeros((G, C), np.float32)
    for c in range(C):
        selT[c // CPG, c] = 1.0
    return dict(wqk=wqk, wv=wv, wp=wp, bqk=bqk, gam=gam, sel=sel,
                selT=selT), resid_bias


def kernel(x, lengths, gamma, beta, Wqkv, bqkv, Wproj, bproj):
    from concourse.bass_utils import run_bass_kernel_spmd

    x = np.asarray(x, np.float32)
    lengths = np.asarray(lengths).astype(np.int64)
    const, resid_bias = _prep_weights(gamma, beta, Wqkv, bqkv, Wproj, bproj)

    in_maps = []
    for s in range(B):
        ln = int(lengths[s])
        mf = (np.arange(L) < ln).astype(np.float32)
        xm = x[s] * mf[:, None]
        xmr = xm + mf[:, None] * resid_bias[None, :]
        xmT = np.ascontiguousarray(xm.T).astype(BF16)
        vmask = mf.reshape(NT, 128).T.copy()
        icnt = np.full((G, 1), 1.0 / max(ln * CPG, 1), np.float32)
        m = dict(const)
        m.update(xmT=xmT, xmr=xmr, vmask=vmask, icnt=icnt)
        in_maps.append(m)

    nc = _get_nc()
    res = run_bass_kernel_spmd(nc, in_maps, core_ids=list(range(B)))
    _CACHE["last_res"] = res
    out = np.stack([res.results[s]["out"] for s in range(B)], axis=0)
    return out.astype(np.float32)


if __name__ == "__main__":
    rng = np.random.default_rng(0)
    x = rng.standard_normal((B, L, C), dtype=np.float32)
    lengths = rng.integers(L // 2, L + 1, size=(B,))
    gamma = np.ones(C, np.float32)
    beta = np.zeros(C, np.float32)
    Wqkv = (rng.standard_normal((C, 3 * C)) * 0.02).astype(np.float32)
    bqkv = np.zeros(3 * C, np.float32)
    Wproj = (rng.standard_normal((C, C)) * 0.02).astype(np.float32)
    bproj = np.zeros(C, np.float32)
    out = kernel(x=x, lengths=lengths, gamma=gamma, beta=beta, Wqkv=Wqkv,
                 bqkv=bqkv, Wproj=Wproj, bproj=bproj)
    print("out", out.shape, out.dtype, np.abs(out).max())


# revision 12
# speedup vs baseline: 1.5013x; 1.0647x over previous
"""Trainium2 Bass kernel for nn_AttentionBlock (masked GroupNorm + jagged full attention).

Contract: kernel(**inputs) takes FULL unsharded inputs (as in reference.setup_inputs())
and returns the FULL [8, 1024, 512] fp32 output. Internally shards data-parallel over
the batch: sample b -> NeuronCore b (8 cores).

v3: fp8e4 (e4m3) DoubleRow matmuls for qkv/v/av/proj (2x contraction per instr, 0.5
cycles/row); bf16 pair-packed scores (2 concurrent PE row groups); exp on ScalarE at
1-ktile granularity with ping-pong PSUM buffers (sA/sB) so the Activation engine (the
roofline: ~1 col/cycle @1.2GHz over H*L*L elements) never stalls; cross-(pair,qc)
software pipelining (the av tail + softmax normalize of pair p run under the scores/exp
of pair p+1); batched DMAs issued from multiple engine queues to cut the launch lead-in.

Per-core dataflow (sample s):
  xmT bf16 [128,CT,L] -> GroupNorm stats (bn_stats + selector matmuls) -> xn8 e4m3
    -> qkT bf16 (DoubleRow fp8, bias on DVE)  -> v8 e4m3 (masked, + denominator row)
    -> per pair: scoresT[k,q] bf16 -> exp e4m3 (ScalarE) -> av^T DoubleRow fp8
    -> reciprocal (DVE) -> partition_broadcast (Pool) -> normalize -> attn8 e4m3
    -> proj DoubleRow fp8 + residual (x*mask + all biases, host-folded), DMA out.

Padded tokens: x host-masked to zero => xn there ~= -mean*rstd*gamma (tiny); v rows
zeroed; denominator mask-row zero. Padded q columns give ~uniform attention over valid
k (|out| ~1e-2, well under the gate); residual rows are zero.
"""

import numpy as np
import ml_dtypes
from contextlib import ExitStack

B, L, C, G, H = 8, 1024, 512, 32, 8
DH = C // H          # 64
CPG = C // G         # 16
EPS = 1e-5
NT = L // 128        # 8 token tiles
CT = C // 128        # 4 channel tiles
CT2 = C // 256       # 2 DoubleRow channel groups
QC = L // 512        # 2 query chunks

BF16 = ml_dtypes.bfloat16
E4M3 = ml_dtypes.float8_e4m3

_CACHE = {}


def _build():
    import concourse.bass as bass
    import concourse.tile as tile
    from concourse import bacc, mybir

    f32 = mybir.dt.float32
    bf16 = mybir.dt.bfloat16
    e4 = mybir.dt.float8e4
    Alu = mybir.AluOpType
    Act = mybir.ActivationFunctionType
    DR = mybir.MatmulPerfMode.DoubleRow

    nc = bacc.Bacc("TRN2", target_bir_lowering=False)

    # ---- per-core DRAM inputs (host-prepped, partition-major for single-descriptor DMA) ----
    xmT_d = nc.dram_tensor("xmT", [128, CT, L], bf16, kind="ExternalInput")
    xm_d = nc.dram_tensor("xmr", [128, NT, C], f32, kind="ExternalInput")  # x*mf + folded biases
    wqk_d = nc.dram_tensor("wqk", [128, CT2, 8, 2, 128], e4, kind="ExternalInput")
    wvp_d = nc.dram_tensor("wvp", [128, 2, CT2, 2, C], e4, kind="ExternalInput")  # [v|p] rhs tiles
    # cs128: gamma[0:4] | bqk[4:12] | vmask[12:20] | sel[20:148]
    cs128_d = nc.dram_tensor("cs128", [128, 20 + CT * G], f32, kind="ExternalInput")
    cs32_d = nc.dram_tensor("cs32", [G, C + 1], f32, kind="ExternalInput")  # selT | icnt
    out_d = nc.dram_tensor("out", [L, C], f32, kind="ExternalOutput")

    with tile.TileContext(nc) as tc, ExitStack() as ctx:
        pc = ctx.enter_context(tc.tile_pool(name="consts", bufs=1))
        pb = ctx.enter_context(tc.tile_pool(name="big", bufs=1))
        ps = ctx.enter_context(tc.tile_pool(name="psum", bufs=1, space="PSUM"))

        # ---- batched loads, spread across engine queues ----
        cs128 = pc.tile([128, 20 + CT * G], f32, tag="cs128", name="cs128")
        cs32 = pc.tile([G, C + 1], f32, tag="cs32", name="cs32")
        nc.scalar.dma_start(cs128[:], cs128_d[:, :])
        nc.scalar.dma_start(cs32[:], cs32_d[:, :])
        xmt_sb = pb.tile([128, CT, L], bf16, tag="xmT", name="xmT")
        nc.sync.dma_start(xmt_sb[:, 0:2, :], xmT_d[:, 0:2, :])
        nc.sync.dma_start(xmt_sb[:, 2:4, :], xmT_d[:, 2:4, :])
        wqk_sb = pc.tile([128, CT2, 8, 2, 128], e4, tag="wqk", name="wqk")
        nc.gpsimd.dma_start(wqk_sb[:], wqk_d[:, :, :, :, :])
        wvp_sb = pc.tile([128, 2, CT2, 2, C], e4, tag="wvp", name="wvp")
        nc.gpsimd.dma_start(wvp_sb[:], wvp_d[:, :, :, :, :])
        xm_sb = pb.tile([128, NT, C], f32, tag="xm", name="xm")
        nc.gpsimd.dma_start(xm_sb[:, 0:4, :], xm_d[:, 0:4, :])
        nc.gpsimd.dma_start(xm_sb[:, 4:8, :], xm_d[:, 4:8, :])

        gam = cs128[:, 0:CT]
        bqk = cs128[:, CT:CT + 8]
        vmask = cs128[:, 12:12 + NT]
        sel = [cs128[:, 20 + G * ct:20 + G * (ct + 1)] for ct in range(CT)]
        selT = cs32[:, 0:C]
        icnt = cs32[:, C:C + 1]

        # ---- Phase 1: GroupNorm (stats over valid tokens; zeros from host masking) ----
        smm = [pb.tile([128, 2], f32, tag=f"smm{t}", name=f"smm{t}") for t in range(CT)]
        ps_g = ps.tile([G, 2], f32, tag="pA", name="psg")
        for t in range(CT):
            bns = pb.tile([128, 2, 6], f32, tag="bns", name="bns")
            nc.vector.bn_stats(bns[:, 0, :], xmt_sb[:, t, 0:512])
            nc.vector.bn_stats(bns[:, 1, :], xmt_sb[:, t, 512:1024])
            mv = pb.tile([128, 2], f32, tag="mv", name="mv")
            nc.vector.bn_aggr(mv[:], bns[:])
            sq = pb.tile([128, 1], f32, tag="sq", name="sq")
            nc.vector.tensor_mul(sq[:], mv[:, 0:1], mv[:, 0:1])
            # smm = [sum(x), sum(x^2)] recovered from mean/var over all 1024 (incl. zeros)
            nc.vector.tensor_scalar(smm[t][:, 0:1], mv[:, 0:1], float(L), None, Alu.mult)
            nc.vector.tensor_scalar(smm[t][:, 1:2], mv[:, 1:2], sq[:, 0:1], float(L), Alu.add, Alu.mult)
        for t in range(CT):
            nc.tensor.matmul(ps_g[:], sel[t], smm[t][:], start=(t == 0), stop=(t == CT - 1))
        grp = pb.tile([G, 2], f32, tag="grp", name="grp")      # [mean_g, rstd_g]
        ex2 = pb.tile([G, 1], f32, tag="ex2", name="ex2")
        nc.vector.tensor_scalar(grp[:, 0:1], ps_g[:, 0:1], icnt, None, Alu.mult)
        nc.vector.tensor_scalar(ex2[:], ps_g[:, 1:2], icnt, None, Alu.mult)
        mm2 = pb.tile([G, 1], f32, tag="mm2", name="mm2")
        nc.vector.tensor_mul(mm2[:], grp[:, 0:1], grp[:, 0:1])
        var = pb.tile([G, 1], f32, tag="var", name="var")
        nc.vector.tensor_tensor(var[:], ex2[:], mm2[:], Alu.subtract)
        sd = pb.tile([G, 1], f32, tag="sd", name="sd")
        eps_sb = pb.tile([G, 1], f32, tag="eps", name="eps")
        nc.vector.memset(eps_sb[:], EPS)
        nc.scalar.activation(sd[:], var[:], Act.Sqrt, bias=eps_sb[:], scale=1.0)
        # preload the Exp activation table while GroupNorm finishes on DVE
        dummy = pb.tile([G, 1], f32, tag="dummy", name="dummy")
        nc.scalar.activation(dummy[:], eps_sb[:], Act.Exp, bias=0.0, scale=1.0)
        nc.vector.reciprocal(grp[:, 1:2], sd[:])

        # xn8[c2][p, i, tok] = xn(channel 256*c2 + 128*i + p, tok) in e4m3
        xn8_sb = [pb.tile([128, 2, L], e4, tag=f"xn8_{c2}", name=f"xn8_{c2}") for c2 in range(CT2)]
        rg_sb = pb.tile([128, CT], f32, tag="rg", name="rg")
        chst = [pb.tile([128, 2], f32, tag=f"chst{t}", name=f"chst{t}") for t in range(CT)]
        for t in range(CT):
            ps_b = ps.tile([128, 2], f32, tag="pB", name="psb")
            nc.tensor.matmul(ps_b[:], selT[:, 128 * t:128 * (t + 1)], grp[:], start=True, stop=True)
            nc.vector.tensor_copy(chst[t][:], ps_b[:])
            nc.vector.tensor_mul(rg_sb[:, t:t + 1], chst[t][:, 1:2], gam[:, t:t + 1])
            nc.vector.tensor_scalar(xn8_sb[t // 2][:, t % 2, :], xmt_sb[:, t, :],
                                    chst[t][:, 0:1], rg_sb[:, t:t + 1],
                                    Alu.subtract, Alu.mult)

        qkT_sb = [pb.tile([128, L], bf16, tag=f"qkT{ot}", name=f"qkT{ot}") for ot in range(8)]
        # v8[p, g, u, h, d]: v of token 128*(2g+u)+p, head h, dh d; d==64 is the mask
        # row; stride 72 keeps the DoubleRow ldweights step 16B-aligned
        v8_sb = pb.tile([128, NT // 2, 2, H, 72], e4, tag="v8", name="v8")
        attn8_sb = [pb.tile([128, 2, L], e4, tag=f"attn8_{c2}", name=f"attn8_{c2}") for c2 in range(CT2)]
        bcast_sb = [pb.tile([64, 512], f32, tag=f"bcast{j}", name=f"bcast{j}", bufs=2) for j in range(2)]

        def emit_qk(ot):
            for qc in range(QC):
                qs = slice(512 * qc, 512 * (qc + 1))
                pq = ps.tile([128, 512], f32, tag=("pA" if qc == 0 else "pB"), name="pq")
                for c2 in range(CT2):
                    nc.tensor.matmul(pq[:], wqk_sb[:, c2, ot, :, :], xn8_sb[c2][:, :, qs],
                                     start=(c2 == 0), stop=(c2 == CT2 - 1), perf_mode=DR)
                nc.vector.tensor_scalar(qkT_sb[ot][:, qs], pq[:],
                                        bqk[:, ot:ot + 1], None, Alu.add)

        def emit_v():
            for kt in range(NT):
                pv = ps.tile([128, 512], f32, tag=("pA" if kt % 2 == 0 else "pB"), name="pv")
                for c2 in range(CT2):
                    nc.tensor.matmul(pv[:], xn8_sb[c2][:, :, 128 * kt:128 * (kt + 1)],
                                     wvp_sb[:, 0, c2, :, :],
                                     start=(c2 == 0), stop=(c2 == CT2 - 1), perf_mode=DR)
                g, u = kt // 2, kt % 2
                nc.vector.tensor_scalar(v8_sb[:, g, u, :, 0:DH],
                                        pv[:].rearrange("p (h d) -> p h d", h=H),
                                        vmask[:, kt:kt + 1], None, Alu.mult)
                nc.vector.tensor_copy(v8_sb[:, g, u, :, DH],
                                      vmask[:, kt:kt + 1].to_broadcast((128, H)))

        def emit_attn(p, qc, fin_prev):
            kT = qkT_sb[4 + p]
            qT = qkT_sb[p]
            qs = slice(512 * qc, 512 * (qc + 1))
            avs = [ps.tile([DH + 1, 512], f32, tag=("avA" if j == 0 else "avB"), name=f"av{j}")
                   for j in range(2)]
            # expT2[p, j, kt, q]: exp'd transposed scores for the two heads of this pair
            expT2 = pb.tile([128, 2, NT, 512], e4, tag="expT2", name="expT2", bufs=2)

            def sc_exp(kt):
                ks = slice(128 * kt, 128 * (kt + 1))
                s = ps.tile([128, 2, 512], f32, tag=("sA" if kt % 2 == 0 else "sB"), name="s")
                nc.tensor.matmul(s[:, 0, :], kT[0:64, ks], qT[0:64, qs], start=True, stop=True)
                nc.tensor.matmul(s[:, 1, :], kT[64:128, ks], qT[64:128, qs], start=True, stop=True)
                nc.scalar.activation(expT2[:, :, kt, :], s[:], Act.Exp, bias=0.0, scale=0.125)

            def av_group(g):
                for j in range(2):
                    h = 2 * p + j
                    nc.tensor.matmul(avs[j][:], v8_sb[:, g, :, h, 0:DH + 1],
                                     expT2[:, j, 2 * g:2 * g + 2, :],
                                     start=(g == 0), stop=(g == NT // 2 - 1), perf_mode=DR)

            # cross-stage software pipeline: finish work of the previous (pair, qc)
            # runs under this pair's first scores/exp; av trails exp by 2 ktiles
            sc_exp(0)
            sc_exp(1)
            if fin_prev is not None:
                fin_prev()
            for kt in range(2, NT):
                sc_exp(kt)
                if kt % 2 == 1:
                    av_group((kt - 3) // 2)   # g = 0,1,2 at kt = 3,5,7

            def finish():
                av_group(NT // 2 - 1)
                for j in range(2):
                    # custom DVE/Pool ops can't read PSUM: copy the denominator row out
                    den = pb.tile([1, 512], f32, tag=f"den{j}", name=f"den{j}", bufs=2)
                    nc.vector.tensor_copy(den[:], avs[j][DH:DH + 1, :])
                    rec = pb.tile([1, 512], f32, tag=f"rec{j}", name=f"rec{j}", bufs=2)
                    nc.vector.reciprocal_approx_fast(rec[:], den[:])
                    nc.gpsimd.partition_broadcast(bcast_sb[j][:], rec[:])
                    nc.vector.tensor_tensor(attn8_sb[p // 2][64 * j:64 * (j + 1), p % 2, qs],
                                            avs[j][0:DH, :], bcast_sb[j][:], Alu.mult)
            return finish

        def emit_proj(qt):
            po = ps.tile([128, 512], f32, tag=("pA" if qt % 2 == 0 else "pB"), name="po")
            for c2 in range(CT2):
                nc.tensor.matmul(po[:], attn8_sb[c2][:, :, 128 * qt:128 * (qt + 1)],
                                 wvp_sb[:, 1, c2, :, :],
                                 start=(c2 == 0), stop=(c2 == CT2 - 1), perf_mode=DR)
            o_sb = pb.tile([128, C], f32, tag=f"o{qt % 2}", name=f"o{qt % 2}")
            nc.vector.tensor_add(o_sb[:], po[:], xm_sb[:, qt, :])
            nc.sync.dma_start(out_d[128 * qt:128 * (qt + 1), :], o_sb[:])

        # ---- emission order: front-load pair 0 so the Activation engine starts early ----
        emit_qk(0)
        emit_qk(4)
        emit_v()
        fin = emit_attn(0, 0, None)
        emit_qk(1)
        emit_qk(5)
        fin = emit_attn(1, 0, fin)
        emit_qk(2)
        emit_qk(6)
        fin = emit_attn(2, 0, fin)
        emit_qk(3)
        emit_qk(7)
        fin = emit_attn(3, 0, fin)
        fin = emit_attn(0, 1, fin)
        emit_proj(0)
        emit_proj(1)
        fin = emit_attn(1, 1, fin)
        emit_proj(2)
        fin = emit_attn(2, 1, fin)
        emit_proj(3)
        fin = emit_attn(3, 1, fin)
        fin()
        for qt in range(NT // 2, NT):
            emit_proj(qt)

    nc.compile()
    return nc


def _get_nc():
    if "nc" not in _CACHE:
        _CACHE["nc"] = _build()
    return _CACHE["nc"]


def _prep_weights(gamma, beta, Wqkv, bqkv, Wproj, bproj):
    """Host-side constant prep shared across cores."""
    W = np.asarray(Wqkv, np.float32)
    bq = np.asarray(bqkv, np.float32) + np.asarray(beta, np.float32) @ W   # fold beta
    Wp = np.asarray(Wproj, np.float32)
    bv = bq[2 * C:3 * C]
    # residual-side constant: bproj + bv @ Wproj (added to masked rows on host)
    resid_bias = np.asarray(bproj, np.float32) + bv @ Wp

    # DoubleRow lhsT/rhs tiles: contraction channel (c2, i, p) = 256*c2 + 128*i + p
    wqk = np.zeros((128, CT2, 8, 2, 128), E4M3)
    wvp = np.zeros((128, 2, CT2, 2, C), E4M3)
    for c2 in range(CT2):
        for i in range(2):
            rows = slice(256 * c2 + 128 * i, 256 * c2 + 128 * (i + 1))
            for ot in range(8):
                wqk[:, c2, ot, i, :] = W[rows, 128 * ot:128 * (ot + 1)].astype(E4M3)
            wvp[:, 0, c2, i, :] = W[rows, 2 * C:3 * C].astype(E4M3)
            wvp[:, 1, c2, i, :] = Wp[rows, :].astype(E4M3)
    cs128 = np.zeros((128, 20 + CT * G), np.float32)
    cs128[:, 0:CT] = np.asarray(gamma, np.float32).reshape(CT, 128).T
    for ot in range(8):
        cs128[:, CT + ot] = bq[128 * ot:128 * (ot + 1)]
    for ct in range(CT):
        for c in range(128):
            cs128[c, 20 + G * ct + (128 * ct + c) // CPG] = 1.0
    cs32 = np.zeros((G, C + 1), np.float32)
    for c in range(C):
        cs32[c // CPG, c] = 1.0
    return dict(wqk=wqk, wvp=wvp), cs128, cs32, resid_bias


def kernel(x, lengths, gamma, beta, Wqkv, bqkv, Wproj, bproj):
    from concourse.bass_utils import run_bass_kernel_spmd

    x = np.asarray(x, np.float32)
    lengths = np.asarray(lengths).astype(np.int64)
    const, cs128_base, cs32, resid_bias = _prep_weights(gamma, beta, Wqkv, bqkv, Wproj, bproj)

    in_maps = []
    for s in range(B):
        ln = int(lengths[s])
        mf = (np.arange(L) < ln).astype(np.float32)
        xm = x[s] * mf[:, None]
        xmr = (xm + mf[:, None] * resid_bias[None, :]).reshape(NT, 128, C).transpose(1, 0, 2)
        xmT = np.ascontiguousarray(xm.T.reshape(CT, 128, L).transpose(1, 0, 2)).astype(BF16)
        cs128 = cs128_base.copy()
        cs128[:, 12:12 + NT] = mf.reshape(NT, 128).T
        cs32_s = cs32.copy()
        cs32_s[:, C] = 1.0 / max(ln * CPG, 1)
        m = dict(const)
        m.update(xmT=xmT, xmr=np.ascontiguousarray(xmr), cs128=cs128, cs32=cs32_s)
        in_maps.append(m)

    nc = _get_nc()
    res = run_bass_kernel_spmd(nc, in_maps, core_ids=list(range(B)))
    _CACHE["last_res"] = res
    out = np.stack([res.results[s]["out"] for s in range(B)], axis=0)
    return out.astype(np.float32)


if __name__ == "__main__":
    rng = np.random.default_rng(0)
    x = rng.standard_normal((B, L, C), dtype=np.float32)
    lengths = rng.integers(L // 2, L + 1, size=(B,))
    gamma = np.ones(C, np.float32)
    beta = np.zeros(C, np.float32)
    Wqkv = (rng.standard_normal((C, 3 * C)) * 0.02).astype(np.float32)
    bqkv = np.zeros(3 * C, np.float32)
    Wproj = (rng.standard_normal((C, C)) * 0.02).astype(np.float32)
    bproj = np.zeros(C, np.float32)
    out = kernel(x=x, lengths=lengths, gamma=gamma, beta=beta, Wqkv=Wqkv,
                 bqkv=bqkv, Wproj=Wproj, bproj=bproj)
    print("out", out.shape, out.dtype, np.abs(out).max())
